# revision 2
# baseline (speedup 1.0000x reference)
"""Trainium2 Bass kernel for nn_LinearPredictionHead (moe_routing).

Reference computation:
    out_e = xs_e[:, :, -1, :] @ W_e + b_e            # [B,C,720] per expert
    combined = sum_e gates[:, e, None] * exp(out_e)  # [B,C,720]
    out = log(max(combined, eps)).transpose(0, 2, 1) # [B,720,C]

Sharding (8 cores, no collectives): 2D data-parallel.
  - B=64 split 4 ways (16 batches -> 512 rows of x per core)
  - P=720 split 2 ways (360 output cols -> W cols per core)
  core c: ib = c // 2 (batch group), ip = c % 2 (p half).

Per-core device kernel, TRANSPOSED layout (r on partitions, p on free):
  psum[r, p] = sum_k x[k, r] * W[k, p]     4 r-tiles x 4 experts, N=360
  (+ b_e[p] via DVE add of an SBUF-broadcast tile for e<3; via an
   in-group rank-1 ones^T x b-row matmul for e=3)
  texp = exp(psum + logg_e[r])             ACT, log-gate as per-partition bias
  acc[rt] += texp                          DVE
  out[rt] = ln(acc[rt])                    ACT, fires per r-tile during the
                                           last expert block; DMA immediately.
This removes the per-group rank-1 log-gate matmuls, the 104-wide runt
tile, and the serialized ln+store tail of the p-major variant.
"""

import os
import sys

import numpy as np

if "/opt/trn_rl_repo" not in sys.path:
    sys.path.insert(0, "/opt/trn_rl_repo")

B, C, E = 64, 32, 4
D, P = 1024, 720
NCORES = 8
BSPLIT, PSPLIT = 4, 2
RB = B // BSPLIT  # 16 batches per core
R = RB * C  # 512 rows per core
RT = 4  # r-tiles of 128
PP = P // PSPLIT  # 360 output cols per core
KO = D // 128  # 8 contraction chunks
EPS = float(np.finfo(np.float64).eps)
NWARM = 6

_CACHE = {}
LAST_RESULT = None


def _build_nc():
    import concourse.tile as tile
    from concourse import bacc, mybir

    f16, f32 = mybir.dt.float16, mybir.dt.float32
    Exp = mybir.ActivationFunctionType.Exp
    Ln = mybir.ActivationFunctionType.Ln
    Copy = mybir.ActivationFunctionType.Copy

    # Force Exp and Ln onto the combined act-table set
    # ("natural_log_exp_and_others", 400 buckets each) so the kernel loads
    # ONE table instead of reloading on every Exp<->Ln switch.
    import concourse.bacc as bacc_mod
    from concourse.hw_specs import get_activation_tables as _orig_gat

    def _patched_gat(arch):
        tables = _orig_gat(arch)
        for name, funcs in tables.items():
            if name != "natural_log_exp_and_others":
                funcs.discard(mybir.ActivationFunctionType.Exp)
                funcs.discard(mybir.ActivationFunctionType.Ln)
        return tables

    bacc_mod.get_activation_tables = _patched_gat

    nc = bacc.Bacc(
        "TRN2", target_bir_lowering=False, debug=False, num_devices=NCORES
    )
    # Host-side layouts pre-tiled for long contiguous DMA runs:
    #   xd[e, ki, rt, ko, rj] = x_e[rt*128+rj, ko*128+ki]   (rt-sliceable, 4KB+)
    #   wd[e, ki, ko, p]      = W_e[ko*128+ki, p]           (2.9-5.8KB runs)
    xd = nc.dram_tensor("xd", [E, 128, RT, KO, 128], f16, kind="ExternalInput").ap()
    wd = nc.dram_tensor("wd", [E, 128, KO, PP], f16, kind="ExternalInput").ap()
    brow = nc.dram_tensor("brow", [1, E * PP], f16, kind="ExternalInput").ap()
    lgb = nc.dram_tensor("lgb", [128, E * RT], f32, kind="ExternalInput").ap()
    # r-major output [R, PP]; host transposes to [RB, PP, C].
    out = nc.dram_tensor("out", [R, PP], f32, kind="ExternalOutput").ap()

    with tile.TileContext(nc) as tc:
        with (
            tc.tile_pool(name="const", bufs=1) as cpool,
            tc.tile_pool(name="psum", bufs=5, space="PSUM") as pspool,
            tc.tile_pool(name="psb", bufs=2, space="PSUM") as psbpool,
            tc.tile_pool(name="texp", bufs=4) as tpool,
            tc.tile_pool(name="lnp", bufs=3) as lnpool,
        ):
            # Warm-up constants via DVE (idle until ~8us in this design).
            warm_t = cpool.tile([128, 512], f16, tag="warm_t")
            nc.vector.memset(warm_t[:], 0.125)
            ones1 = cpool.tile([1, 128], f16, tag="ones")
            nc.vector.memset(ones1[:], 1.0)

            # PE warm-up: dep-free FULL-ARRAY matmuls fill the initial
            # DMA-wait window so the HAM clock gate reaches 8/8 before real
            # work starts.
            warm_ps = pspool.tile([128, 512], f32, tag="warm", bufs=1)
            for _ in range(NWARM):
                nc.tensor.matmul(
                    warm_ps[:, :],
                    warm_t[:, :128],
                    warm_t[:, :],
                    start=True,
                    stop=True,
                )

            # Tiny tensors on the scalar (ACT) HWDGE ring so they don't
            # queue behind the big streaming loads on the sync ring.
            lgbt = cpool.tile([128, E * RT], f32, tag="lgbt")
            nc.scalar.dma_start(lgbt[:], lgb[:, :])
            browt = cpool.tile([1, E * PP], f16, tag="browt")
            nc.scalar.dma_start(browt[:], brow[:, :])

            # b_bcast prep for experts 0..2: rank-1 ones^T x b-row into
            # PSUM, then ACT-copy to SBUF. Expert 3 adds b via an in-group
            # rank-1 instead (keeps its exp->add->ln->store chain short).
            bb = cpool.tile([128, 3, PP], f32, tag="bb")
            for e in range(3):
                ps_b = psbpool.tile([128, PP], f32, tag="psb")
                nc.tensor.matmul(
                    ps_b[:, :],
                    ones1[:, :],
                    browt[:, e * PP : (e + 1) * PP],
                    start=True,
                    stop=True,
                )
                nc.scalar.activation(bb[:, e, :], ps_b[:, :], Copy)

            # Streaming loads on the sync ring, expert-major to match
            # compute order. First chunks are small so the first matmul
            # group's deps land early.
            xs, ws = [], []
            for e in range(E):
                xe = cpool.tile([128, RT, KO, 128], f16, tag=f"x{e}")
                we = cpool.tile([128, KO, PP], f16, tag=f"w{e}")
                xs.append(xe)
                ws.append(we)
                if e == 0:
                    nc.sync.dma_start(we[:, :2], wd[e, :, :2])
                    nc.sync.dma_start(xe[:, :1], xd[e, :, :1])
                    nc.sync.dma_start(we[:, 2:5], wd[e, :, 2:5])
                    nc.sync.dma_start(we[:, 5:], wd[e, :, 5:])
                    nc.sync.dma_start(xe[:, 1:], xd[e, :, 1:])
                else:
                    nc.sync.dma_start(we[:, :4], wd[e, :, :4])
                    nc.sync.dma_start(xe[:, :2], xd[e, :, :2])
                    nc.sync.dma_start(we[:, 4:], wd[e, :, 4:])
                    nc.sync.dma_start(xe[:, 2:], xd[e, :, 2:])

            accs = [None] * RT
            for e in range(E):
                for rt in range(RT):
                    ps = pspool.tile([128, PP], f32, tag="ps")
                    for ko in range(KO):
                        nc.tensor.matmul(
                            ps[:, :],
                            xs[e][:, rt, ko, :],
                            ws[e][:, ko, :],
                            start=(ko == 0),
                            stop=(ko == KO - 1 and e != 3),
                        )
                    bias_ap = lgbt[:, e * RT + rt : e * RT + rt + 1]
                    if e == 0:
                        # psum += b_0 broadcast; exp writes acc directly.
                        nc.vector.tensor_add(ps[:, :], ps[:, :], bb[:, e, :])
                        acc = cpool.tile([128, PP], f32, tag=f"acc{rt}")
                        nc.scalar.activation(acc[:, :], ps[:, :], Exp, bias=bias_ap)
                        accs[rt] = acc
                    elif e < 3:
                        nc.vector.tensor_add(ps[:, :], ps[:, :], bb[:, e, :])
                        te = tpool.tile([128, PP], f32, tag="texp", name="te")
                        nc.scalar.activation(te[:, :], ps[:, :], Exp, bias=bias_ap)
                        acc = accs[rt]
                        nc.vector.tensor_add(acc[:, :], acc[:, :], te[:, :])
                    else:
                        # e == 3: b via in-group rank-1, then the output
                        # chain fires per r-tile (overlaps later groups).
                        nc.tensor.matmul(
                            ps[:, :],
                            ones1[:, :],
                            browt[:, 3 * PP : 4 * PP],
                            start=False,
                            stop=True,
                        )
                        acc = accs[rt]
                        # Final r-tile: split the post-chain in halves so
                        # the very last exp->add->ln->store is short.
                        splits = (
                            [(0, PP // 2), (PP // 2, PP)] if rt == RT - 1 else [(0, PP)]
                        )
                        ln_t = lnpool.tile([128, PP], f32, tag="ln")
                        for lo, hi in splits:
                            te = tpool.tile([128, PP], f32, tag="texp", name="te")
                            nc.scalar.activation(
                                te[:, lo:hi], ps[:, lo:hi], Exp, bias=bias_ap
                            )
                            nc.vector.tensor_add(
                                acc[:, lo:hi], acc[:, lo:hi], te[:, lo:hi]
                            )
                            nc.scalar.activation(ln_t[:, lo:hi], acc[:, lo:hi], Ln)
                            nc.sync.dma_start(
                                out[rt * 128 : (rt + 1) * 128, lo:hi],
                                ln_t[:, lo:hi],
                            )

    nc.compile()
    return nc


def _prep_inputs(inputs):
    gates = np.asarray(inputs["gates"], dtype=np.float32)
    Ws = [np.asarray(inputs[f"W{i}"], dtype=np.float32) for i in range(E)]
    bs = [np.asarray(inputs[f"b{i}"], dtype=np.float32) for i in range(E)]

    # Per p-half: wd[e, ki, ko, p] = W_e[ko*128+ki, ip*PP+p]
    wd_halves = []
    brow_halves = []
    for ip in range(PSPLIT):
        wts = []
        for e in range(E):
            wh = Ws[e][:, ip * PP : (ip + 1) * PP].astype(np.float16)  # [D, PP]
            wts.append(wh.reshape(KO, 128, PP).transpose(1, 0, 2))
        wd_halves.append(np.ascontiguousarray(np.stack(wts)))  # [E,128,KO,PP]
        br = np.concatenate([bs[e][ip * PP : (ip + 1) * PP] for e in range(E)])
        brow_halves.append(br.reshape(1, E * PP).astype(np.float16))

    # Per b-group: xd[e, ki, rt, ko, rj] = x_e[rt*128+rj, ko*128+ki]
    xd_groups = []
    lgb_groups = []
    for ib in range(BSPLIT):
        xts = []
        for e in range(E):
            xl = np.asarray(inputs[f"xs{e}"][ib * RB : (ib + 1) * RB, :, -1, :])
            x2 = xl.reshape(R, D).astype(np.float16)  # [R, D]
            xts.append(
                np.ascontiguousarray(
                    x2.reshape(RT, 128, KO, 128).transpose(3, 0, 2, 1)
                )
            )
        xd_groups.append(np.stack(xts))  # [E, 128, RT, KO, 128]
        g = gates[ib * RB : (ib + 1) * RB, :]  # [RB, E]
        lgv = np.log(np.maximum(g.astype(np.float64), 1e-30))  # [RB, E]
        lg = np.empty((128, E * RT), np.float32)
        for e in range(E):
            rep = np.repeat(lgv[:, e], C)  # [R]
            lg[:, e * RT : (e + 1) * RT] = rep.reshape(RT, 128).T
        lgb_groups.append(lg)

    in_maps = []
    for c in range(NCORES):
        ib, ip = divmod(c, PSPLIT)
        in_maps.append(
            {
                "xd": xd_groups[ib],
                "wd": wd_halves[ip],
                "brow": brow_halves[ip],
                "lgb": lgb_groups[ib],
            }
        )
    return in_maps


def _install_trace_support():
    """Dev-only plumbing for NTFF profiling under axon: provides the
    antenv.axon_hooks shim this image lacks and disables the S3 artifact
    upload. Returns True if tracing is usable."""
    try:
        import types

        import antenv

        if "antenv.axon_hooks" not in sys.modules:
            mod = types.ModuleType("antenv.axon_hooks")
            mod._hook = None

            def set_axon_ntff_profile_hook(h, _m=mod):
                _m._hook = h

            def get_axon_ntff_profile_hook(_m=mod):
                return _m._hook

            mod.set_axon_ntff_profile_hook = set_axon_ntff_profile_hook
            mod.get_axon_ntff_profile_hook = get_axon_ntff_profile_hook
            sys.modules["antenv.axon_hooks"] = mod
            antenv.axon_hooks = mod

        import antenv.axon_hooks as ah

        if ah.get_axon_ntff_profile_hook() is None:
            from trn_agent_boot.trn_boot import _ntff_profile_via_ctypes

            hook = _ntff_profile_via_ctypes("/opt/axon/libaxon_pjrt.so")
            if hook is None:
                return False
            ah.set_axon_ntff_profile_hook(hook)

        import concourse.bass_utils as bu

        bu.upload_artifacts = lambda tmpdir: f"local:{tmpdir}"
        return True
    except Exception as e:  # pragma: no cover - tracing is best-effort
        print(f"trace support unavailable: {type(e).__name__}: {e}")
        return False


def kernel(**inputs):
    global LAST_RESULT
    from concourse.bass_utils import run_bass_kernel_spmd

    if "nc" not in _CACHE:
        _CACHE["nc"] = _build_nc()
    nc = _CACHE["nc"]

    in_maps = _prep_inputs(inputs)
    trace = os.environ.get("BASS_KERNEL_TRACE", "0") == "1"
    if trace:
        trace = _install_trace_support()
    res = run_bass_kernel_spmd(
        nc, in_maps, core_ids=list(range(NCORES)), trace=trace
    )
    LAST_RESULT = res

    out = np.empty((B, P, C), np.float32)
    for c in range(NCORES):
        ib, ip = divmod(c, PSPLIT)
        # device output is r-major [R, PP]
        out[ib * RB : (ib + 1) * RB, ip * PP : (ip + 1) * PP, :] = (
            res.results[c]["out"].reshape(RB, C, PP).transpose(0, 2, 1)
        )
    return out


# revision 4
# speedup vs baseline: 1.1142x; 1.1142x over previous
"""Trainium2 Bass kernel for nn_LinearPredictionHead (moe_routing).

Reference computation:
    out_e = xs_e[:, :, -1, :] @ W_e + b_e            # [B,C,720] per expert
    combined = sum_e gates[:, e, None] * exp(out_e)  # [B,C,720]
    out = log(max(combined, eps)).transpose(0, 2, 1) # [B,720,C]

Sharding (8 cores, no collectives): 2D data-parallel.
  - B=64 split 4 ways (16 batches -> 512 rows of x per core)
  - P=720 split 2 ways (360 output cols -> W cols per core)
  core c: ib = c // 2 (batch group), ip = c % 2 (p half).

Per-core device kernel (p-major, N=512 streams hide LDWEIGHTS):
  psum[p, r] = sum_k W[k, p] * x[k, r]     12 groups (e, p-tile), N=512
  te  = exp(psum + b_e[p])                 ACT, per-partition bias
  acc += te * g_bcast_e                    DVE mul+add; gate broadcast tiles
                                           are built once by 4 rank-1s
  (for the last group (e3,p2) the gate rides the PSUM as a rank-1 log-g
   matmul so the final chain is exp->add->ln->store, no mul)
  out[p_i] = ln(acc[p_i])                  fires per p-tile during the e3
                                           block; DMA'd immediately.
Startup: the first x/W chunks go out via GPSIMD/SWDGE (its queue clears
the NEFF preamble ~3us before the sync ring), and warm-up matmuls keep
the PE busy from ~5us so the HAM clock gate is open when real work lands.
"""

import os
import sys

import numpy as np

if "/opt/trn_rl_repo" not in sys.path:
    sys.path.insert(0, "/opt/trn_rl_repo")

B, C, E = 64, 32, 4
D, P = 1024, 720
NCORES = 8
BSPLIT, PSPLIT = 4, 2
RB = B // BSPLIT  # 16 batches per core
R = RB * C  # 512 rows per core
PP = P // PSPLIT  # 360 output cols per core
PTS = [(0, 128), (128, 128), (256, 104)]  # p-tiles within PP
KO = D // 128  # 8 contraction chunks
EPS = float(np.finfo(np.float64).eps)
NWARM = 5

_CACHE = {}
LAST_RESULT = None


def _build_nc():
    import concourse.tile as tile
    from concourse import bacc, mybir

    f16, f32 = mybir.dt.float16, mybir.dt.float32
    Exp = mybir.ActivationFunctionType.Exp
    Ln = mybir.ActivationFunctionType.Ln

    # Force Exp and Ln onto the combined act-table set so the kernel loads
    # ONE table instead of reloading on every Exp<->Ln switch.
    import concourse.bacc as bacc_mod
    from concourse.hw_specs import get_activation_tables as _orig_gat

    def _patched_gat(arch):
        tables = _orig_gat(arch)
        for name, funcs in tables.items():
            if name != "natural_log_exp_and_others":
                funcs.discard(mybir.ActivationFunctionType.Exp)
                funcs.discard(mybir.ActivationFunctionType.Ln)
        return tables

    bacc_mod.get_activation_tables = _patched_gat

    nc = bacc.Bacc(
        "TRN2", target_bir_lowering=False, debug=False, num_devices=NCORES
    )
    # Host-side layouts pre-tiled for long contiguous DMA runs:
    #   xd[e, ki, ko, r]  = x_e[r, ko*128+ki]        (8KB runs/partition)
    #   wd[e, ki, ko, p]  = W_e[ko*128+ki, p]        (5.76KB runs/partition)
    xd = nc.dram_tensor("xd", [E, 128, KO, R], f16, kind="ExternalInput").ap()
    wd = nc.dram_tensor("wd", [E, 128, KO, PP], f16, kind="ExternalInput").ap()
    grow = nc.dram_tensor("grow", [1, E * R], f16, kind="ExternalInput").ap()
    lgrow = nc.dram_tensor("lgrow", [1, R], f16, kind="ExternalInput").ap()
    bias = nc.dram_tensor("bias", [128, E * 3], f32, kind="ExternalInput").ap()
    # p-major output (contiguous 2KB DMA runs); host transposes to [RB,PP,C].
    out = nc.dram_tensor("out", [PP, RB, C], f32, kind="ExternalOutput").ap()

    with tile.TileContext(nc) as tc:
        with (
            tc.tile_pool(name="const", bufs=1) as cpool,
            tc.tile_pool(name="psum", bufs=4, space="PSUM") as pspool,
            tc.tile_pool(name="psg", bufs=2, space="PSUM") as psgpool,
            tc.tile_pool(name="texp", bufs=5) as tpool,
            tc.tile_pool(name="lnp", bufs=3) as lnpool,
        ):
            # Constants via GPSIMD (its queue clears the preamble first, and
            # this keeps DVE/ACT free).
            warm_t = cpool.tile([128, 512], f16, tag="warm_t")
            nc.gpsimd.memset(warm_t[:], 0.125)
            ones1 = cpool.tile([1, 128], f16, tag="ones")
            nc.gpsimd.memset(ones1[:], 1.0)

            # First compute chunks via SWDGE: gpsimd dispatches ~3.5us into
            # the NEFF, ~4us before the sync ring's first DIRECT2D.
            xs, ws = [], []
            for e in range(E):
                xs.append(
                    cpool.tile([128, KO, R], f16, tag=f"x{e}", name=f"x{e}")
                )
                ws.append(
                    cpool.tile([128, KO, PP], f16, tag=f"w{e}", name=f"w{e}")
                )
            h = KO // 2
            nc.gpsimd.dma_start(xs[0][:, :h, :], xd[0, :, :h, :])
            nc.gpsimd.dma_start(ws[0][:, :2], wd[0, :, :2])

            # Small tensors on the scalar (ACT) HWDGE ring.
            growt = cpool.tile([1, E * R], f16, tag="growt")
            nc.scalar.dma_start(growt[:], grow[:, :])
            lgrowt = cpool.tile([1, R], f16, tag="lgrowt")
            nc.scalar.dma_start(lgrowt[:], lgrow[:, :])
            bias_t = cpool.tile([128, E * 3], f32, tag="bias")
            nc.scalar.dma_start(bias_t[:], bias[:, :])

            # Remaining stream on the sync ring, expert-major, ko-halves
            # interleaved so each block can start on its first half.
            nc.sync.dma_start(ws[0][:, 2:5], wd[0, :, 2:5])
            nc.sync.dma_start(xs[0][:, h:, :], xd[0, :, h:, :])
            nc.sync.dma_start(ws[0][:, 5:], wd[0, :, 5:])
            for e in range(1, E):
                nc.sync.dma_start(ws[e][:, :h], wd[e, :, :h])
                nc.sync.dma_start(xs[e][:, :h, :], xd[e, :, :h, :])
                nc.sync.dma_start(ws[e][:, h:], wd[e, :, h:])
                nc.sync.dma_start(xs[e][:, h:, :], xd[e, :, h:, :])

            # PE warm-up: dep-free matmuls bridge the DMA-wait window so the
            # HAM clock gate is at 8/8 when real matmuls start.
            warm_ps = pspool.tile([128, 512], f32, tag="warm", bufs=1)
            for _ in range(NWARM):
                nc.tensor.matmul(
                    warm_ps[:, :],
                    warm_t[:, :128],
                    warm_t[:, :],
                    start=True,
                    stop=True,
                )

            def mm_group(e, p_i, with_lg_rank1=False):
                p0, plen = PTS[p_i]
                ps = pspool.tile([128, 512], f32, tag="ps")
                for ko in range(KO):
                    nc.tensor.matmul(
                        ps[:plen, :],
                        ws[e][:, ko, p0 : p0 + plen],
                        xs[e][:, ko, :],
                        start=(ko == 0),
                        stop=(ko == KO - 1 and not with_lg_rank1),
                    )
                if with_lg_rank1:
                    # += ones.T @ log(g_e3): folds the gate into the exp so
                    # the final chain needs no DVE multiply.
                    nc.tensor.matmul(
                        ps[:plen, :],
                        ones1[:, :plen],
                        lgrowt[:, :],
                        start=False,
                        stop=True,
                    )
                return ps

            # e0 matmul groups first (PE gets real work ASAP) ...
            e0_ps = [mm_group(0, p_i) for p_i in range(3)]

            # ... then gate-broadcast prep (warm by now): 4 rank-1s + DVE
            # copies build g_bcast[e] = [128, R] tiles.
            gbs = []
            for e in range(E):
                psg = psgpool.tile([128, 512], f32, tag="psg")
                nc.tensor.matmul(
                    psg[:, :],
                    ones1[:, :],
                    growt[:, e * R : (e + 1) * R],
                    start=True,
                    stop=True,
                )
                gb = cpool.tile([128, R], f32, tag=f"gb{e}")
                nc.vector.tensor_copy(gb[:, :], psg[:, :])
                gbs.append(gb)

            accs = [None] * 3

            def chain(e, p_i, ps, last=False):
                p0, plen = PTS[p_i]
                bias_ap = bias_t[:plen, e * 3 + p_i : e * 3 + p_i + 1]
                te = tpool.tile([128, 512], f32, tag="te", name="te")
                nc.scalar.activation(te[:plen, :], ps[:plen, :], Exp, bias=bias_ap)
                if e == 0:
                    acc = cpool.tile([128, 512], f32, tag=f"acc{p_i}")
                    nc.vector.tensor_mul(acc[:plen, :], te[:plen, :], gbs[0][:plen, :])
                    accs[p_i] = acc
                else:
                    acc = accs[p_i]
                    if last:
                        # gate already in psum via the log-g rank-1
                        nc.vector.tensor_add(acc[:plen, :], acc[:plen, :], te[:plen, :])
                    else:
                        tg = tpool.tile([128, 512], f32, tag="te", name="tg")
                        nc.vector.tensor_mul(
                            tg[:plen, :], te[:plen, :], gbs[e][:plen, :]
                        )
                        nc.vector.tensor_add(acc[:plen, :], acc[:plen, :], tg[:plen, :])
                if e == E - 1:
                    ln_t = lnpool.tile([128, 512], f32, tag="ln")
                    nc.scalar.activation(ln_t[:plen, :], acc[:plen, :], Ln)
                    nc.sync.dma_start(
                        out[p0 : p0 + plen].rearrange("p b c -> p (b c)"),
                        ln_t[:plen, :],
                    )

            for p_i in range(3):
                chain(0, p_i, e0_ps[p_i])
            for e in range(1, E):
                for p_i in range(3):
                    is_last = e == E - 1 and p_i == 2
                    ps = mm_group(e, p_i, with_lg_rank1=is_last)
                    chain(e, p_i, ps, last=is_last)

    nc.compile()
    return nc


def _prep_inputs(inputs):
    gates = np.asarray(inputs["gates"], dtype=np.float32)
    Ws = [np.asarray(inputs[f"W{i}"], dtype=np.float32) for i in range(E)]
    bs = [np.asarray(inputs[f"b{i}"], dtype=np.float32) for i in range(E)]

    # Per p-half: wd[e, ki, ko, p] = W_e[ko*128+ki, ip*PP+p]
    wd_halves = []
    bias_halves = []
    for ip in range(PSPLIT):
        wts = []
        for e in range(E):
            wh = Ws[e][:, ip * PP : (ip + 1) * PP].astype(np.float16)
            wts.append(wh.reshape(KO, 128, PP).transpose(1, 0, 2))
        wd_halves.append(np.ascontiguousarray(np.stack(wts)))
        bt = np.zeros((128, E * 3), np.float32)
        for e in range(E):
            for p_i, (p0, plen) in enumerate(PTS):
                bt[:plen, e * 3 + p_i] = bs[e][ip * PP + p0 : ip * PP + p0 + plen]
        bias_halves.append(bt)

    # Per b-group: xd[e, ki, ko, r] = x_e[r, ko*128+ki]; gate rows.
    xd_groups = []
    grow_groups = []
    lgrow_groups = []
    for ib in range(BSPLIT):
        xts = []
        for e in range(E):
            xl = np.asarray(inputs[f"xs{e}"][ib * RB : (ib + 1) * RB, :, -1, :])
            x2 = xl.reshape(R, D).astype(np.float16)
            xts.append(
                np.ascontiguousarray(x2.reshape(R, KO, 128).transpose(2, 1, 0))
            )
        xd_groups.append(np.stack(xts))  # [E, 128, KO, R]
        g = gates[ib * RB : (ib + 1) * RB, :]  # [RB, E]
        grow = np.concatenate(
            [np.repeat(g[:, e], C) for e in range(E)]
        )  # [E*R]
        grow_groups.append(grow.reshape(1, E * R).astype(np.float16))
        lgv = np.log(np.maximum(g[:, E - 1].astype(np.float64), 1e-30))
        lgrow_groups.append(
            np.repeat(lgv, C).reshape(1, R).astype(np.float16)
        )

    in_maps = []
    for c in range(NCORES):
        ib, ip = divmod(c, PSPLIT)
        in_maps.append(
            {
                "xd": xd_groups[ib],
                "wd": wd_halves[ip],
                "grow": grow_groups[ib],
                "lgrow": lgrow_groups[ib],
                "bias": bias_halves[ip],
            }
        )
    return in_maps


def _install_trace_support():
    """Dev-only plumbing for NTFF profiling under axon: provides the
    antenv.axon_hooks shim this image lacks and disables the S3 artifact
    upload. Returns True if tracing is usable."""
    try:
        import types

        import antenv

        if "antenv.axon_hooks" not in sys.modules:
            mod = types.ModuleType("antenv.axon_hooks")
            mod._hook = None

            def set_axon_ntff_profile_hook(h, _m=mod):
                _m._hook = h

            def get_axon_ntff_profile_hook(_m=mod):
                return _m._hook

            mod.set_axon_ntff_profile_hook = set_axon_ntff_profile_hook
            mod.get_axon_ntff_profile_hook = get_axon_ntff_profile_hook
            sys.modules["antenv.axon_hooks"] = mod
            antenv.axon_hooks = mod

        import antenv.axon_hooks as ah

        if ah.get_axon_ntff_profile_hook() is None:
            from trn_agent_boot.trn_boot import _ntff_profile_via_ctypes

            hook = _ntff_profile_via_ctypes("/opt/axon/libaxon_pjrt.so")
            if hook is None:
                return False
            ah.set_axon_ntff_profile_hook(hook)

        import concourse.bass_utils as bu

        bu.upload_artifacts = lambda tmpdir: f"local:{tmpdir}"
        return True
    except Exception as e:  # pragma: no cover - tracing is best-effort
        print(f"trace support unavailable: {type(e).__name__}: {e}")
        return False


def kernel(**inputs):
    global LAST_RESULT
    from concourse.bass_utils import run_bass_kernel_spmd

    if "nc" not in _CACHE:
        _CACHE["nc"] = _build_nc()
    nc = _CACHE["nc"]

    in_maps = _prep_inputs(inputs)
    trace = os.environ.get("BASS_KERNEL_TRACE", "0") == "1"
    if trace:
        trace = _install_trace_support()
    res = run_bass_kernel_spmd(
        nc, in_maps, core_ids=list(range(NCORES)), trace=trace
    )
    LAST_RESULT = res

    out = np.empty((B, P, C), np.float32)
    for c in range(NCORES):
        ib, ip = divmod(c, PSPLIT)
        # device output is p-major [PP, RB, C]
        out[ib * RB : (ib + 1) * RB, ip * PP : (ip + 1) * PP, :] = res.results[c][
            "out"
        ].transpose(1, 0, 2)
    return out
